# revision 3
# baseline (speedup 1.0000x reference)
"""CoAtNet transformer block on 8 trn2 NeuronCores, data-parallel over batch.

v2: ACT-bound design. Attention in qq-quarters (196 q-cols) so each kt's
scores fit one 2-bank PSUM tile -> one exp call per kt. fp8-e4m3 DoubleRow
matmuls for QKV/V/out-proj/AV/denominator/FFN2 (weights host-scaled x64),
bf16 for scores and FFN1. pS = exp(s)*exp(b-4.5) in fp8 (shift keeps fp8 in
range; softmax shift-invariant). FFN/out-proj/QKV-of-next-batch emitted as
"filler" work spliced between attention groups so TensorE stays busy (and
the HAM clock warm) while ACT grinds the exps.

PSUM budget: scores 2x2 banks + av/dn 1 + fillers 2+1 = 8.
"""

import os
import sys

import numpy as np
import ml_dtypes

sys.path.insert(0, "/opt/trn_rl_repo")

H, W, C, HEADS = 28, 28, 512, 16
N = H * W            # 784
FF = 4 * C           # 2048
DH = C // HEADS      # 32
B = 16
NCORES = 8
BPC = B // NCORES    # 2
P = 128
NMT = C // P         # 4
NKT = 7              # token tiles (6x128 + 16)
NFT = FF // P        # 16
QH = N // 2          # 392
QQ = N // 4          # 196
MCOLS = 1552
EPS = 1e-5
SC = 64.0            # fp8 weight scale
KSH = 4.5            # exp shift folded into bias master

bf16 = ml_dtypes.bfloat16
f8np = ml_dtypes.float8_e4m3


def _tok(kt):
    return P if kt < NKT - 1 else N - (NKT - 1) * P  # 128 or 16


def _build_master(rel_bias: np.ndarray) -> np.ndarray:
    """exp(b - KSH) Toeplitz strips, as in the baseline master."""
    padded = np.zeros((HEADS, 1708), np.float32)
    padded[:, : rel_bias.shape[1]] = rel_bias
    e = np.exp(padded - KSH)
    idx = 1580 + np.arange(P)[:, None] - np.arange(MCOLS)[None, :]
    return np.ascontiguousarray(e[:, idx]).astype(bf16)


def _build_nc():
    import concourse.bass as bass  # noqa: F401
    import concourse.mybir as mybir
    import concourse.tile as tile
    from concourse import bacc
    from concourse.masks import make_identity

    fp32 = mybir.dt.float32
    bfl = mybir.dt.bfloat16
    f8 = mybir.dt.float8e4
    ALU = mybir.AluOpType
    AF = mybir.ActivationFunctionType
    DR = mybir.MatmulPerfMode.DoubleRow
    NODR = bool(int(os.environ.get("KV2_NODR", "0")))

    def mm_pair(ps, lhsT_pair, rhs_pair, start, stop, lhs_sl=None, rhs_sl=None):
        """One DoubleRow matmul over a k-subtile pair, or two plain matmuls
        when KV2_NODR=1 (HW bring-up fallback)."""
        if not NODR:
            nc.tensor.matmul(ps, lhsT_pair, rhs_pair, start=start, stop=stop,
                             perf_mode=DR)
        else:
            for s in range(2):
                nc.tensor.matmul(ps, lhsT_pair[:, s], rhs_pair[:, s],
                                 start=(start and s == 0),
                                 stop=(stop and s == 1))

    nc = bacc.Bacc("TRN2", target_bir_lowering=False, debug=False)

    xin = nc.dram_tensor("xin", (BPC, C, N), bfl, kind="ExternalInput").ap()
    wq = nc.dram_tensor("wq", (C, C), f8, kind="ExternalInput").ap()
    wk = nc.dram_tensor("wk", (C, C), f8, kind="ExternalInput").ap()
    wv = nc.dram_tensor("wv", (C, C), f8, kind="ExternalInput").ap()
    wo = nc.dram_tensor("wo", (C, C), f8, kind="ExternalInput").ap()
    bq = nc.dram_tensor("bq", (C,), fp32, kind="ExternalInput").ap()
    bk = nc.dram_tensor("bk", (C,), fp32, kind="ExternalInput").ap()
    w1 = nc.dram_tensor("w1", (C, FF), bfl, kind="ExternalInput").ap()
    b1 = nc.dram_tensor("b1", (FF,), fp32, kind="ExternalInput").ap()
    w2 = nc.dram_tensor("w2", (FF, C), f8, kind="ExternalInput").ap()
    b2 = nc.dram_tensor("b2", (C,), fp32, kind="ExternalInput").ap()
    expe = nc.dram_tensor("expe", (HEADS, P, MCOLS), bfl, kind="ExternalInput").ap()
    out = nc.dram_tensor("out", (BPC, C, N), fp32, kind="ExternalOutput").ap()

    x_t = xin.rearrange("b (mt p) n -> mt p b n", p=P)
    out_t = out.rearrange("b (mt p) n -> mt p b n", p=P)

    with tile.TileContext(nc) as tc:
        const = tc.alloc_tile_pool(name="const", bufs=1)
        act = tc.alloc_tile_pool(name="act", bufs=1)

        # ---- persistent SBUF ---------------------------------------------
        xT = [act.tile([P, BPC, N], bfl, tag=f"xT{m}", name=f"xT{m}") for m in range(NMT)]
        for m in range(NMT):
            nc.sync.dma_start(xT[m][:], x_t[m])

        wq8 = const.tile([P, NMT, C], f8, tag="wq8", name="wq8")
        wk8 = const.tile([P, NMT, C], f8, tag="wk8", name="wk8")
        wv8 = const.tile([P, NMT, C], f8, tag="wv8", name="wv8")
        wo8 = const.tile([P, NMT, C], f8, tag="wo8", name="wo8")
        for w_d, w_s in ((wq, wq8), (wk, wk8), (wv, wv8), (wo, wo8)):
            nc.sync.dma_start(w_s[:], w_d.rearrange("(ks p) m -> p ks m", p=P))
        eG = const.tile([P, HEADS, MCOLS], bfl, tag="eG", name="eG")
        nc.sync.dma_start(eG[:], expe.rearrange("h p c -> p h c"))
        w1S = const.tile([P, NMT, FF], bfl, tag="w1S", name="w1S")
        nc.sync.dma_start(w1S[:], w1.rearrange("(ks p) m -> p ks m", p=P))
        w28 = const.tile([P, NFT, C], f8, tag="w28", name="w28")
        nc.sync.dma_start(w28[:], w2.rearrange("(ks p) m -> p ks m", p=P))

        bqS = const.tile([P, NMT], fp32, tag="bqS", name="bqS")
        bkS = const.tile([P, NMT], fp32, tag="bkS", name="bkS")
        b2S = const.tile([P, NMT], fp32, tag="b2S", name="b2S")
        for b_d, b_s in ((bq, bqS), (bk, bkS), (b2, b2S)):
            nc.sync.dma_start(b_s[:], b_d.rearrange("(mt p) -> p mt", p=P))
        b1S = const.tile([P, NFT], fp32, tag="b1S", name="b1S")
        nc.sync.dma_start(b1S[:], b1.rearrange("(mt p) -> p mt", p=P))

        ones_bf = const.tile([P, DH], bfl, tag="ones_bf", name="ones_bf")
        nc.any.memset(ones_bf[:], 1.0)
        ones8 = const.tile([P, 2, DH], f8, tag="ones8", name="ones8")
        nc.any.memset(ones8[:], 1.0)

        xn8 = act.tile([P, NMT, BPC, N], f8, tag="xn8", name="xn8")
        qT = [act.tile([P, BPC, N], bfl, tag=f"qT{m}", name=f"qT{m}") for m in range(NMT)]
        kT = [act.tile([P, BPC, N], bfl, tag=f"kT{m}", name=f"kT{m}") for m in range(NMT)]
        vS8 = [act.tile([P, NKT, C], bfl, tag=f"vS8{b}", name=f"vS8{b}") for b in range(BPC)]
        cat8 = act.tile([P, NMT, BPC, N], f8, tag="cat8", name="cat8")
        x1T = [act.tile([P, BPC, N], bfl, tag=f"x1T{m}", name=f"x1T{m}") for m in range(NMT)]

        # ---- LayerNorm ----------------------------------------------------
        with tc.tile_pool(name="lnp", bufs=1) as lnp, \
             tc.tile_pool(name="lnps", bufs=2, space="PSUM") as lnps:
            xsq = [lnp.tile([P, BPC, N], bfl, tag=f"xsq{m}", name=f"xsq{m}") for m in range(NMT)]
            for m in range(NMT):
                nc.scalar.square(xsq[m][:], xT[m][:])
            must = lnp.tile([1, BPC, N], fp32, tag="must", name="must")
            sqst = lnp.tile([1, BPC, N], fp32, tag="sqst", name="sqst")
            for ch in range(4):
                b_i, h_i = ch // 2, ch % 2
                sl = (slice(None), b_i, slice(h_i * QH, (h_i + 1) * QH))
                sp = lnps.tile([P, 512], fp32)
                for ks in range(NMT):
                    nc.tensor.matmul(sp[0:1, :QH], ones_bf[:, 0:1], xT[ks][sl],
                                     start=(ks == 0), stop=(ks == NMT - 1),
                                     tile_position=(0, 0))
                    nc.tensor.matmul(sp[32:33, :QH], ones_bf[:, 0:1], xsq[ks][sl],
                                     start=(ks == 0), stop=(ks == NMT - 1),
                                     tile_position=(0, 32))
                nc.vector.tensor_scalar_mul(must[0:1, b_i, sl[2]], sp[0:1, :QH], 1.0 / C)
                nc.vector.tensor_scalar_mul(sqst[0:1, b_i, sl[2]], sp[32:33, :QH], 1.0 / C)
            mu = must[:]
            msq = sqst[:]
            t1 = lnp.tile([1, BPC, N], fp32, tag="t1", name="t1")
            t2 = lnp.tile([1, BPC, N], fp32, tag="t2", name="t2")
            nc.vector.tensor_mul(t1[:], mu, mu)
            # t2 = var = (msq + eps) - mu^2
            nc.vector.scalar_tensor_tensor(t2[:], msq, float(EPS), t1[:],
                                           ALU.add, ALU.subtract)
            nc.scalar.activation(t1[:], t2[:], AF.Sqrt)       # t1 = sd
            nc.vector.reciprocal_approx_accurate(t2[:], t1[:], sqst[:])  # t2 = 1/sd
            nc.vector.scalar_tensor_tensor(t1[:], mu, -1.0, t2[:],
                                           ALU.mult, ALU.mult)  # t1 = -mu/sd
            rsig_bf = lnp.tile([1, BPC, N], bfl, tag="rsig_bf", name="rsig_bf")
            negmur_bf = lnp.tile([1, BPC, N], bfl, tag="negmur_bf", name="negmur_bf")
            nc.vector.tensor_copy(rsig_bf[:], t2[:])
            nc.vector.tensor_copy(negmur_bf[:], t1[:])
            rsigB = lnp.tile([P, BPC, N], bfl, tag="rsigB", name="rsigB")
            negmurB = lnp.tile([P, BPC, N], bfl, tag="negmurB", name="negmurB")
            nc.gpsimd.partition_broadcast(rsigB[:], rsig_bf[:])
            nc.gpsimd.partition_broadcast(negmurB[:], negmur_bf[:])
            for m in range(NMT):
                nc.vector.tensor_mul(xsq[m][:], xT[m][:], rsigB[:])
                nc.vector.tensor_add(xn8[:, m], xsq[m][:], negmurB[:])

        # ---- filler machinery --------------------------------------------
        # Two 1-bank PSUM pools, alternated so filler matmul groups are
        # double-buffered. All matmul outputs stay bank-aligned (offset 0).
        filA = tc.alloc_tile_pool(name="filA", bufs=1, space="PSUM")
        filB = tc.alloc_tile_pool(name="filB", bufs=1, space="PSUM")
        _pp = [0]

        def fil_alt():
            _pp[0] ^= 1
            if _pp[0]:
                return filA.tile([P, 512], fp32, tag="fa", name="fa")
            return filB.tile([P, 512], fp32, tag="fb", name="fb")

        def qk_group(wS, bS, dst, m, b_i, qh):
            qsl = slice(qh * QH, (qh + 1) * QH)
            ps = fil_alt()
            for t in range(2):
                mm_pair(ps[:, :QH],
                        wS[:, 2 * t:2 * t + 2, m * P:(m + 1) * P],
                        xn8[:, 2 * t:2 * t + 2, b_i, qsl],
                        t == 0, t == 1)
            nc.vector.tensor_scalar(dst[m][:, b_i, qsl], ps[:, :QH], 1.0 / SC,
                                    bS[:, m:m + 1], ALU.mult, ALU.add)

        def v_group(b_i, kt):
            tok = _tok(kt)
            ksl = slice(kt * P, kt * P + tok)
            ps = fil_alt()
            for t in range(2):
                mm_pair(ps[:tok, :], xn8[:, 2 * t:2 * t + 2, b_i, ksl],
                        wv8[:, 2 * t:2 * t + 2, :], t == 0, t == 1)
            nc.vector.tensor_scalar_mul(vS8[b_i][:tok, kt, :], ps[:tok, :], 1.0 / SC)

        def op_group(m, b_i, qh):
            # NOTE: no bf16 ident-residual matmul here — mixing bf16 and fp8
            # instructions in one PSUM accumulation group misbehaves on HW.
            # bo is folded into b1/b2 host-side; residual added on DVE.
            qsl = slice(qh * QH, (qh + 1) * QH)
            ps = fil_alt()
            for t in range(2):
                mm_pair(ps[:, :QH], wo8[:, 2 * t:2 * t + 2, m * P:(m + 1) * P],
                        cat8[:, 2 * t:2 * t + 2, b_i, qsl], t == 0, t == 1)
            nc.vector.scalar_tensor_tensor(x1T[m][:, b_i, qsl], ps[:, :QH],
                                           1.0 / SC, xT[m][:, b_i, qsl],
                                           ALU.mult, ALU.add)

        fftp = tc.alloc_tile_pool(name="fftp", bufs=1)
        fft_cur = {}

        def f1_burst(b_i, qh):
            if b_i not in fft_cur:
                fft_cur[b_i] = fftp.tile([P, NFT, N], f8, tag="fft", name="fft")
            fft = fft_cur[b_i]
            qsl = slice(qh * QH, (qh + 1) * QH)
            for mf in range(NFT):
                ps = fil_alt()
                for ks in range(NMT):
                    nc.tensor.matmul(ps[:, :QH], w1S[:, ks, mf * P:(mf + 1) * P],
                                     x1T[ks][:, b_i, qsl],
                                     start=(ks == 0), stop=(ks == NMT - 1))
                nc.scalar.activation(fft[:, mf, qsl], ps[:, :QH], AF.Gelu,
                                     bias=b1S[:, mf:mf + 1])

        outp = tc.alloc_tile_pool(name="outp", bufs=2)

        def f2_group(m, b_i, qh):
            fft = fft_cur[b_i]
            qsl = slice(qh * QH, (qh + 1) * QH)
            ps = fil_alt()
            for t in range(NFT // 2):
                mm_pair(ps[:, :QH], w28[:, 2 * t:2 * t + 2, m * P:(m + 1) * P],
                        fft[:, 2 * t:2 * t + 2, qsl], t == 0, t == NFT // 2 - 1)
            o2 = outp.tile([P, QH], fp32, tag="o2", name="o2")
            nc.vector.tensor_scalar(o2[:], ps[:, :QH], 1.0 / SC, b2S[:, m:m + 1],
                                    ALU.mult, ALU.add)
            nc.vector.tensor_add(o2[:], o2[:], x1T[m][:, b_i, qsl])
            nc.sync.dma_start(out_t[m][:, b_i, qsl], o2[:])

        FILLERS = []

        def drain(k):
            n = 0
            while FILLERS and n < k:
                FILLERS.pop(0)()
                n += 1

        # ---- QKV for b0 (direct) -----------------------------------------
        for m in range(NMT):
            for qh in range(2):
                qk_group(wq8, bqS, qT, m, 0, qh)
                qk_group(wk8, bkS, kT, m, 0, qh)
        for kt in range(NKT):
            v_group(0, kt)

        TR = int(os.environ.get("KV2_TRUNC", "0"))
        if TR == 0:
            # queue QKV(b1) as filler for b0's attention
            for m in range(NMT):
                for qh in range(2):
                    FILLERS.append(lambda m=m, qh=qh: qk_group(wq8, bqS, qT, m, 1, qh))
                    FILLERS.append(lambda m=m, qh=qh: qk_group(wk8, bkS, kT, m, 1, qh))
            for kt in range(NKT):
                FILLERS.append(lambda kt=kt: v_group(1, kt))

        # ---- attention ----------------------------------------------------
        scp = tc.alloc_tile_pool(name="scp", bufs=2, space="PSUM")
        avp = tc.alloc_tile_pool(name="avp", bufs=1, space="PSUM")
        dnp = tc.alloc_tile_pool(name="dnp", bufs=1, space="PSUM")
        expp = tc.alloc_tile_pool(name="expp", bufs=4)
        psp = tc.alloc_tile_pool(name="psp", bufs=4)
        rp = tc.alloc_tile_pool(name="rp", bufs=2)

        def attn_group(b_i, qh, g):
            # baseline-style layout: all matmul outs bank-aligned.
            qsl = slice(qh * QH, (qh + 1) * QH)
            av = avp.tile([P, 512], fp32, tag="av", name="av")
            dn = dnp.tile([P, 512], fp32, tag="dn", name="dn")

            def emit_avdn(ptiles, kt):
                tok = _tok(kt)
                for j in range(4):
                    hp, jj = j // 2, j % 2
                    csl = slice(P * g + 32 * j, P * g + 32 * j + 32)
                    nc.tensor.matmul(av[32 * j:32 * j + 32, :QH],
                                     vS8[b_i][:tok, kt, csl],
                                     ptiles[hp][:tok, jj, :],
                                     start=(kt == 0), stop=(kt == NKT - 1),
                                     tile_position=(0, 32 * j),
                                     skip_group_check=True)
                    nc.tensor.matmul(dn[32 * j:32 * j + 32, :QH],
                                     ones_bf[:tok, :],
                                     ptiles[hp][:tok, jj, :],
                                     start=(kt == 0), stop=(kt == NKT - 1),
                                     tile_position=(0, 32 * j),
                                     skip_group_check=True)

            pend = None
            for kt in range(NKT):
                tok = _tok(kt)
                ksl = slice(kt * P, kt * P + tok)
                scH = [scp.tile([P, 2, 512], fp32, tag="sc", name="sc")
                       for _ in range(2)]
                for j in range(4):
                    nc.tensor.matmul(scH[j // 2][:tok, j % 2, :QH],
                                     kT[g][32 * j:32 * j + 32, b_i, ksl],
                                     qT[g][32 * j:32 * j + 32, b_i, qsl],
                                     start=True, stop=True,
                                     tile_position=(32 * j, 0))
                c0 = 768 - P * kt + qh * QH
                ptiles = []
                for hp in range(2):
                    es = expp.tile([P, 2, QH], bfl, tag="es", name="es")
                    nc.scalar.activation(es[:tok], scH[hp][:tok, :, :QH], AF.Exp)
                    p8 = psp.tile([P, 2, QH], bfl, tag="p8", name="p8")
                    nc.vector.tensor_mul(
                        p8[:tok], es[:tok],
                        eG[:tok, 4 * g + 2 * hp:4 * g + 2 * hp + 2, c0:c0 + QH])
                    ptiles.append(p8)
                if pend is not None:
                    emit_avdn(*pend)
                pend = (ptiles, kt)
            emit_avdn(*pend)
            recipB = rp.tile([P, QH], fp32, tag="rb", name="rb")
            nc.vector.reciprocal_approx_fast(recipB[:], dn[:, :QH])
            nc.vector.tensor_mul(cat8[:, g, b_i, qsl], av[:, :QH], recipB[:])

        for b_i in range(BPC if TR == 0 else (0 if TR == 1 else 1)):
            for qh in range(2):
                for g in range(4):
                    attn_group(b_i, qh, g)
                    if TR == 0:
                        drain(4)
                if TR != 0:
                    continue
                # queue out-proj / FFN as their inputs complete. The qkv(b1)
                # fillers were queued first; FIFO draining retires them well
                # before b1's attention reads qT/kT/vS8[1].
                for m in range(NMT):
                    FILLERS.append(lambda m=m, b=b_i, q=qh: op_group(m, b, q))
        drain(len(FILLERS))
        # FFN as a sequential tail: keeps all gelus contiguous on ACT (two
        # table switches total) and off the attention exp stream.
        if TR == 0:
            for b_i in range(BPC):
                # both f1 bursts first: f1(qh1) matmuls overlap qh0's gelu
                # drain instead of stalling behind f2(qh0) in the queue.
                f1_burst(b_i, 0)
                f1_burst(b_i, 1)
                for qh in range(2):
                    for m in range(NMT):
                        f2_group(m, b_i, qh)

        for p in (rp, psp, expp, dnp, avp, scp, outp, fftp,
                  filB, filA, act, const):
            p.release()

    nc.compile()
    return nc


_NC_CACHE = None


def kernel(**inputs) -> np.ndarray:
    global _NC_CACHE
    x = np.asarray(inputs["x"], np.float32)
    ln_w = np.asarray(inputs["ln_w"], np.float32)
    ln_b = np.asarray(inputs["ln_b"], np.float32)
    Wq = np.asarray(inputs["Wq"], np.float32)
    Wk = np.asarray(inputs["Wk"], np.float32)
    Wv = np.asarray(inputs["Wv"], np.float32)
    Wo = np.asarray(inputs["Wo"], np.float32)
    bq = np.asarray(inputs["bq"], np.float32)
    bk = np.asarray(inputs["bk"], np.float32)
    bv = np.asarray(inputs["bv"], np.float32)
    bo = np.asarray(inputs["bo"], np.float32)
    rel_bias = np.asarray(inputs["rel_bias"], np.float32)
    W1 = np.asarray(inputs["W1"], np.float32)
    b1 = np.asarray(inputs["b1"], np.float32)
    W2 = np.asarray(inputs["W2"], np.float32)
    b2 = np.asarray(inputs["b2"], np.float32)

    # fold LayerNorm affine into projections (exact)
    Wq_f = ln_w[:, None] * Wq
    Wk_f = ln_w[:, None] * Wk
    Wv_f = ln_w[:, None] * Wv
    bq_f = ln_b @ Wq + bq
    bk_f = ln_b @ Wk + bk
    bv_f = ln_b @ Wv + bv
    bo_f = bo + bv_f @ Wo
    b1_f = b1 + bo_f @ W1
    b2_f = b2 + bo_f

    master = _build_master(rel_bias)

    def q8(w):
        return np.ascontiguousarray((w * SC).astype(f8np))

    nc = _NC_CACHE
    if nc is None:
        nc = _build_nc()
        _NC_CACHE = nc

    shared = {
        "wq": q8(Wq_f), "wk": q8(Wk_f), "wv": q8(Wv_f), "wo": q8(Wo),
        "bq": bq_f, "bk": bk_f,
        "w1": np.ascontiguousarray(W1.astype(bf16)), "b1": b1_f,
        "w2": q8(W2), "b2": b2_f,
        "expe": master,
    }
    xr = x.reshape(B, C, N)
    in_maps = []
    for c in range(NCORES):
        m = dict(shared)
        m["xin"] = np.ascontiguousarray(xr[c * BPC:(c + 1) * BPC].astype(bf16))
        in_maps.append(m)

    from concourse.bass_utils import run_bass_kernel_spmd

    res = run_bass_kernel_spmd(
        nc, in_maps, core_ids=list(range(NCORES)),
        trace=bool(int(os.environ.get("KERNEL_TRACE", "0"))),
        tmpdir=os.environ.get("KERNEL_TRACE_DIR") or None,
    )
    if res.exec_time_ns is not None:
        print(f"HW exec time: {res.exec_time_ns} ns", file=sys.stderr)
    outs = [r["out"].reshape(BPC, C, H, W) for r in res.results]
    return np.concatenate(outs, axis=0).astype(np.float32)


if __name__ == "__main__":
    _build_nc()
    print("build ok")
